# revision 3
# baseline (speedup 1.0000x reference)
"""CTLSTM (Neural Hawkes continuous-time LSTM) Trainium2 kernel.

Data-parallel over batch across 8 NeuronCores (8 batch rows per core).
Per core, the T=200 recurrence runs serially; per step the h@Wh matmul is
done as G^T = Wh^T h^T (output laid out hidden-unit-major across the 128
partitions) so all elementwise gate math runs at full DVE/ACT width.

Numerics tricks (validated host-side, max abs err ~0.02 on absmax ~5.3):
  - Wh/Wx/x/h in bf16, fp32 PSUM accumulation; gx (x@Wx+b for all steps)
    precomputed on-chip, stored bf16.
  - Single ACT LUT table (natural_log_exp: Exp/Ln/Identity) for the whole
    program -> no 1.3us table reloads. sigmoid/tanh are built from exp +
    DVE reciprocal; softplus = Ln(exp(g)+1) via activation bias.
  - Weight columns pre-scaled so ONE exp op covers all 7 gates:
    sigma-gates * -1, z * -2, d * +1; Wh additionally negated because the
    h produced on-chip is -h (sign absorbed by the tanh/STT identities).
"""

import os
from contextlib import ExitStack

import numpy as np
import ml_dtypes

import concourse.bass as bass
import concourse.bacc as bacc
import concourse.mybir as mybir
import concourse.tile as tile
from concourse.bass_utils import run_bass_kernel_spmd

BF16 = ml_dtypes.bfloat16

B, T, D, H = 64, 200, 256, 512
NCORES = 8
BL = B // NCORES          # 8 batch rows per core
G7 = 7 * H                # 3584 gate columns
NM = G7 // 128            # 28 M-tiles
KH = H // 128             # 4 K-tiles for Wh
KD = D // 128             # 2 K-tiles for Wx
NTB = T * BL              # 1600 (t, b) pairs
RING = 16                 # output ring slots (statesize 128 each)
DMA_EVERY = 8

# new gate order (i, ib, f, fb, o, z, d) -> original split order
# (gi, gf, gz, go, gib, gfb, gd)
GATE_PERM = [0, 4, 1, 5, 3, 2, 6]
COL_SCALE = [-1.0, -1.0, -1.0, -1.0, -1.0, -2.0, 1.0]

F32 = mybir.dt.float32
BF = mybir.dt.bfloat16
AF = mybir.ActivationFunctionType
OP = mybir.AluOpType

_PROGRAM_CACHE = {}


class _OneTableBacc(bacc.Bacc):
    """Pin every activation to the natural_log_exp_and_others LUT table.

    The stock table-placement pass commits to the first table containing
    each func, so an Exp/Exp/Exp/Ln-per-step program flip-flops between the
    exp table and the ln table — 2 table loads x 1.28us per step. All our
    funcs (Exp, Ln, Identity) live together in natural_log_exp_and_others,
    so blank out every other table and the pass emits exactly one load.
    """

    def insert_act_table_loads(self):
        from concourse.hw_specs import get_activation_tables

        has_activation = any(
            isinstance(i, mybir.InstActivation)
            for b in self.main_func.blocks
            for i in b.instructions
        )
        if not has_activation:
            return
        keep = "natural_log_exp_and_others"
        tables = [
            (n, (s if n == keep else set()))
            for n, s in get_activation_tables(self.m.arch).items()
        ]
        bacc._bass_rust.insert_act_table_loads(self, tables)


def _build_program():
    nc = _OneTableBacc("TRN2", target_bir_lowering=False, debug=False)

    whs_d = nc.dram_tensor("whs", [128, KH * G7], BF, kind="ExternalInput").ap()
    wxs_d = nc.dram_tensor("wxs", [128, KD * G7], BF, kind="ExternalInput").ap()
    xts_d = nc.dram_tensor("xts", [128, KD * NTB], BF, kind="ExternalInput").ap()
    ndt_d = nc.dram_tensor("negdt", [128, T * 4 * BL], F32, kind="ExternalInput").ap()
    bcol_d = nc.dram_tensor("bcol", [128, NM], F32, kind="ExternalInput").ap()
    out_d = nc.dram_tensor("outs", [128, T * 128], F32, kind="ExternalOutput").ap()
    out_r = out_d.rearrange("p (t s) -> p t s", s=128)

    with tile.TileContext(nc) as tc, ExitStack() as ctx:
        const = ctx.enter_context(tc.tile_pool(name="const", bufs=1))
        whs = const.tile([128, KH * G7], BF, tag="whs")
        wxs = const.tile([128, KD * G7], BF, tag="wxs")
        xts = const.tile([128, KD * NTB], BF, tag="xts")
        ndt = const.tile([128, T * 4 * BL], F32, tag="ndt")
        bcol = const.tile([128, NM], F32, tag="bcol")
        gx = const.tile([128, T * 224], BF, tag="gx")
        ring = const.tile([128, RING * 128], F32, tag="ring")

        nc.sync.dma_start(whs[:], whs_d)
        nc.sync.dma_start(wxs[:], wxs_d)
        nc.sync.dma_start(xts[:], xts_d)
        nc.sync.dma_start(ndt[:], ndt_d)
        nc.sync.dma_start(bcol[:], bcol_d)

        gx_r = gx.rearrange("p (t g) -> p t g", g=224)
        ndt_r = ndt.rearrange("p (t x) -> p t x", x=4 * BL)
        ring_r = ring.rearrange("p (s x) -> p s x", x=128)

        # zero-init states: step 0 reads ring slot RING-1
        nc.vector.memset(ring_r[:, RING - 1, :], 0.0)

        # ---- phase 1: gx[t] = (x_t @ Wx' + b')^T for all t, bf16 ----
        nchunks = []
        n0 = 0
        while n0 < NTB:
            nsz = min(512, NTB - n0)
            nchunks.append((n0, nsz))
            n0 += nsz
        with tc.tile_pool(name="gxps", bufs=2, space="PSUM") as gxps:
            ci = 0
            for m in range(NM):
                for (n0, nsz) in nchunks:
                    ps = gxps.tile([128, 512], F32, tag="gxp")
                    for k in range(KD):
                        nc.tensor.matmul(
                            ps[:, :nsz],
                            wxs[:, k * G7 + m * 128 : k * G7 + (m + 1) * 128],
                            xts[:, k * NTB + n0 : k * NTB + n0 + nsz],
                            start=(k == 0),
                            stop=(k == KD - 1),
                        )
                    t0, nt = n0 // BL, nsz // BL
                    src = ps[:, :nsz].rearrange("p (t b) -> p t b", b=BL)
                    dst = gx_r[:, t0 : t0 + nt, m * BL : (m + 1) * BL]
                    if ci % 2 == 0:
                        nc.scalar.activation(dst, src, AF.Identity, bias=bcol[:, m : m + 1])
                    else:
                        nc.vector.tensor_scalar(dst, src, bcol[:, m : m + 1], None, OP.add)
                    ci += 1

        # ---- phase 2: the recurrence ----
        sp = ctx.enter_context(tc.tile_pool(name="sp", bufs=3))
        hp = ctx.enter_context(tc.tile_pool(name="hp", bufs=3))
        psp = ctx.enter_context(tc.tile_pool(name="psp", bufs=2, space="PSUM"))

        for t in range(T):
            slot = t % RING
            prev = (t - 1) % RING
            c_p = ring_r[:, prev, 0:32]
            cb_p = ring_r[:, prev, 32:64]
            dl_p = ring_r[:, prev, 64:96]
            o_p = ring_r[:, prev, 96:128]

            # decay: c_d = cbar + (c - cbar) * exp(-delta*dt)
            e_in = sp.tile([128, 32], F32, tag="e_in")
            nc.vector.tensor_mul(e_in[:], ndt_r[:, t, :], dl_p)
            E = sp.tile([128, 32], F32, tag="E")
            nc.scalar.activation(E[:], e_in[:], AF.Exp)
            cmb = sp.tile([128, 32], F32, tag="cmb")
            nc.vector.tensor_sub(cmb[:], c_p, cb_p)
            cmbE = sp.tile([128, 32], F32, tag="cmbE")
            nc.vector.tensor_mul(cmbE[:], cmb[:], E[:])
            CD = sp.tile([128, 64], F32, tag="CD")  # [c_d | cbar]
            nc.vector.tensor_add(CD[:, 0:32], cmbE[:], cb_p)
            nc.vector.tensor_copy(CD[:, 32:64], cb_p)

            # h' = -o*tanh(c_d) = o*(u_c-1)/(1+u_c), u_c = exp(-2 c_d)
            u_c = sp.tile([128, 32], F32, tag="u_c")
            nc.scalar.activation(u_c[:], CD[:, 0:32], AF.Exp, scale=-2.0)
            v_c = sp.tile([128, 32], F32, tag="v_c")
            nc.vector.tensor_scalar_add(v_c[:], u_c[:], 1.0)
            r_c = sp.tile([128, 32], F32, tag="r_c")
            nc.vector.reciprocal(r_c[:], v_c[:])
            w_c = sp.tile([128, 32], F32, tag="w_c")
            nc.vector.scalar_tensor_tensor(w_c[:], u_c[:], 1.0, o_p, OP.subtract, OP.mult)
            h = hp.tile([128, 4 * BL], BF, tag="h")
            nc.vector.tensor_mul(h[:], w_c[:], r_c[:])

            # G^T += Wh'^T h'^T  (112 small matmuls, LDW-bound, FWL bf16)
            ps = psp.tile([128, 224], F32, tag="gps")
            for m in range(NM):
                for k in range(KH):
                    nc.tensor.matmul(
                        ps[:, m * BL : (m + 1) * BL],
                        whs[:, k * G7 + m * 128 : k * G7 + (m + 1) * 128],
                        h[:, k * BL : (k + 1) * BL],
                        start=(k == 0),
                        stop=(k == KH - 1),
                    )

            Gs = sp.tile([128, 224], F32, tag="Gs")
            nc.vector.tensor_add(Gs[:], ps[:], gx_r[:, t, :])
            u = sp.tile([128, 224], F32, tag="u")
            nc.scalar.activation(u[:], Gs[:], AF.Exp)
            v = sp.tile([128, 192], F32, tag="v")
            nc.vector.tensor_scalar_add(v[:], u[:, 0:192], 1.0)
            sA = sp.tile([128, 128], F32, tag="sA")  # [i | ib | f | fb]
            nc.vector.reciprocal(sA[:], v[:, 0:128])
            nc.vector.reciprocal(ring_r[:, slot, 96:128], v[:, 128:160])  # o
            r_z = sp.tile([128, 32], F32, tag="r_z")
            nc.vector.reciprocal(r_z[:], v[:, 160:192])
            # delta = softplus(gd) = ln(u_d + 1)
            nc.scalar.activation(ring_r[:, slot, 64:96], u[:, 192:224], AF.Ln, bias=1.0)
            # zz = (u_z - 1)*r_z = -z ; TI = [i*zz | ib*zz] = [-i*z | -ib*z]
            zz = sp.tile([128, 32], F32, tag="zz")
            nc.vector.scalar_tensor_tensor(zz[:], u[:, 160:192], 1.0, r_z[:], OP.subtract, OP.mult)
            TI = sp.tile([128, 64], F32, tag="TI")
            nc.vector.tensor_mul(TI[:, 0:32], sA[:, 0:32], zz[:])
            nc.vector.tensor_mul(TI[:, 32:64], sA[:, 32:64], zz[:])
            P2 = sp.tile([128, 64], F32, tag="P2")  # [f*c_d | fb*cbar]
            nc.vector.tensor_mul(P2[:], sA[:, 64:128], CD[:])
            # c_new = f*c_d + i*z ; cbar_new = fb*cbar + ib*z
            nc.vector.tensor_sub(ring_r[:, slot, 0:64], P2[:], TI[:])

            if t % DMA_EVERY == DMA_EVERY - 1:
                lo = slot - (DMA_EVERY - 1)
                nc.sync.dma_start(
                    out_r[:, t - (DMA_EVERY - 1) : t + 1, :],
                    ring_r[:, lo : slot + 1, :],
                )

    nc.compile()
    return nc


def _get_program():
    if "nc" not in _PROGRAM_CACHE:
        _PROGRAM_CACHE["nc"] = _build_program()
    return _PROGRAM_CACHE["nc"]


def _prep_shared(Wx, Wh, b):
    perm = np.concatenate([g * H + np.arange(H) for g in GATE_PERM])
    scale = np.repeat(np.array(COL_SCALE, np.float32), H)
    WxP = (Wx[:, perm] * scale).astype(np.float32)
    WhP = (-(Wh[:, perm] * scale)).astype(np.float32)
    bP = (b[perm] * scale).astype(np.float32)
    whs = np.ascontiguousarray(
        WhP.reshape(KH, 128, G7).transpose(1, 0, 2).reshape(128, KH * G7)
    ).astype(BF16)
    wxs = np.ascontiguousarray(
        WxP.reshape(KD, 128, G7).transpose(1, 0, 2).reshape(128, KD * G7)
    ).astype(BF16)
    bcol = np.ascontiguousarray(bP.reshape(NM, 128).T).astype(np.float32)
    return whs, wxs, bcol


def make_in_maps(input_, duration, Wx, Wh, b):
    X = np.asarray(input_, np.float32)
    dur = np.asarray(duration, np.float32)
    whs, wxs, bcol = _prep_shared(
        np.asarray(Wx, np.float32), np.asarray(Wh, np.float32), np.asarray(b, np.float32)
    )
    in_maps = []
    for ci in range(NCORES):
        Xc = X[ci * BL : (ci + 1) * BL]              # (BL, T, D)
        xts = np.ascontiguousarray(
            Xc.transpose(2, 1, 0).reshape(KD, 128, NTB).transpose(1, 0, 2).reshape(128, KD * NTB)
        ).astype(BF16)
        ndc = -dur[ci * BL : (ci + 1) * BL].T        # (T, BL)
        negdt = np.ascontiguousarray(
            np.broadcast_to(ndc[None, :, None, :], (128, T, 4, BL)).reshape(128, T * 4 * BL)
        ).astype(np.float32)
        in_maps.append(
            {"whs": whs, "wxs": wxs, "xts": xts, "negdt": negdt, "bcol": bcol}
        )
    return in_maps


def assemble_output(results):
    full = np.empty((4, B, T, H), np.float32)
    for ci in range(NCORES):
        arr = np.asarray(results[ci]["outs"]).reshape(128, T, 4, 4, BL)
        # arr[p, t, state, hc, b] -> full[state, b, t, hc*128 + p]
        full[:, ci * BL : (ci + 1) * BL] = (
            arr.transpose(2, 4, 1, 3, 0).reshape(4, BL, T, H)
        )
    return full


def kernel(**inputs):
    nc = _get_program()
    in_maps = make_in_maps(
        inputs["input_"], inputs["duration"], inputs["Wx"], inputs["Wh"], inputs["b"]
    )
    res = run_bass_kernel_spmd(nc, in_maps, list(range(NCORES)))
    return assemble_output(res.results)


def run_traced(**inputs):
    """Like kernel() but also returns exec_time_ns from the NTFF profile."""
    nc = _get_program()
    in_maps = make_in_maps(
        inputs["input_"], inputs["duration"], inputs["Wx"], inputs["Wh"], inputs["b"]
    )
    res = run_bass_kernel_spmd(nc, in_maps, list(range(NCORES)), trace=True)
    return assemble_output(res.results), res
